# revision 1
# baseline (speedup 1.0000x reference)
"""Trainium2 Bass kernel for DeepseekV3 naive MoE (expert-parallel over 8 cores).

Contract: kernel(**inputs) takes FULL unsharded numpy inputs
(hidden_states [T,H] f32, top_k_index [T,K] i32, top_k_weights [T,K] f32,
wg [E,H,I] f32, wu [E,H,I] f32, wd [E,I,H] f32) and returns the FULL
[T,H] f32 output, equal to the reference grouped-GEMM MoE.

Strategy (hardcoded for T=8192, H=1024, I=1408, E=32, K=8, 8 cores):
 - Host: replicate tokens K times, stable-sort pairs by expert id, pad each
   expert's group to a shared capacity cap = ceil128(max group size) (the
   device program is compiled and cached per cap), build transposed
   activations xT [H, cap] per expert (contraction dim on partitions), cast
   matmul operands to bf16 (PSUM accumulation stays fp32).
 - Device (SPMD, 4 experts per core): for each expert, for each 512-column
   chunk: gateT = wg.T @ xT, upT = wu.T @ xT (PSUM f32 accumulate over H),
   actT = silu(gateT) * upT (bf16), downT = wd.T @ actT, scaled by the
   per-pair router weight, stored as [H, CAP] per expert.
 - Host: transpose back, unsort, sum the K weighted contributions per token.
"""

import os
import sys

for _p in ("/opt/trn_rl_repo", "/root/.axon_site/_ro/trn_rl_repo"):
    if _p not in sys.path:
        sys.path.insert(0, _p)

# recover wedged NeuronCores on first touch; NEFF cache across rounds
os.environ.setdefault("NEURON_RT_RESET_CORES", "1")
os.environ.setdefault("MYCRO_LOCAL_CACHE", "1")

import numpy as np
import ml_dtypes

import concourse.bass as bass  # noqa: F401  (registers types)
import concourse.tile as tile
from concourse import bacc, mybir

# Problem dims (fixed by the task)
E, H, I, K, T = 32, 1024, 1408, 8, 8192
N_CORES = 8
EL = E // N_CORES  # experts per core
P = 128
HO, IO = H // P, I // P  # 8, 11
CHUNK = 512
CAP_QUANTUM = 128


def _chunks_of(cap: int):
    out = []
    off = 0
    while off < cap:
        w = min(CHUNK, cap - off)
        out.append((off, w))
        off += w
    return out

BF16 = ml_dtypes.bfloat16

_CACHE: dict = {}


def _build_nc(cap: int, repeat: int = 1):
    """Build + compile the per-core Bass kernel for per-expert capacity `cap`.

    repeat>1 duplicates the whole schedule in-kernel (same IO); used only to
    amortize launch overhead when measuring device execution time."""
    assert cap % CAP_QUANTUM == 0
    chunk_list = _chunks_of(cap)
    dt_mm = mybir.dt.bfloat16

    nc = bacc.Bacc("TRN2", target_bir_lowering=False, debug=False)

    xT_d = nc.dram_tensor("xT", [EL, HO, P, cap], dt_mm, kind="ExternalInput")
    wg_d = nc.dram_tensor("wg", [EL, HO, P, I], dt_mm, kind="ExternalInput")
    wu_d = nc.dram_tensor("wu", [EL, HO, P, I], dt_mm, kind="ExternalInput")
    wd_d = nc.dram_tensor("wd", [EL, IO, P, H], dt_mm, kind="ExternalInput")
    wr_d = nc.dram_tensor("wr", [EL, P, cap], mybir.dt.float32, kind="ExternalInput")
    out_d = nc.dram_tensor("out", [EL, HO, P, cap], mybir.dt.float32, kind="ExternalOutput")

    sched = [(e, off, w) for e in range(EL) for (off, w) in chunk_list] * repeat

    with tile.TileContext(nc) as tc:
        with (
            tc.tile_pool(name="wpool", bufs=2) as wpool,
            tc.tile_pool(name="wdpool", bufs=2) as wdpool,
            tc.tile_pool(name="xpool", bufs=3) as xpool,
            tc.tile_pool(name="apool", bufs=2) as apool,
            tc.tile_pool(name="opool", bufs=3) as opool,
            tc.tile_pool(name="rpool", bufs=2) as rpool,
            tc.tile_pool(name="gps", bufs=2, space="PSUM") as gps,
            tc.tile_pool(name="ups", bufs=2, space="PSUM") as ups,
            tc.tile_pool(name="dps", bufs=3, space="PSUM") as dps,
        ):
            wtiles = {}  # live weight tiles for current expert
            act_tiles = {}  # chunk index -> act tile
            x_live = {}

            def emit_gu(j):
                e, off, w = sched[j]
                if off == 0:
                    wgt = wpool.tile([P, HO, I], dt_mm, tag="wg")
                    for ho in range(HO):
                        nc.sync.dma_start(wgt[:, ho, :], wg_d[e, ho])
                    wut = wpool.tile([P, HO, I], dt_mm, tag="wu")
                    for ho in range(HO):
                        nc.sync.dma_start(wut[:, ho, :], wu_d[e, ho])
                    wdt = wdpool.tile([P, IO, H], dt_mm, tag="wd")
                    for io in range(IO):
                        nc.sync.dma_start(wdt[:, io, :], wd_d[e, io])
                    wrt = rpool.tile([P, cap], mybir.dt.float32, tag="wr")
                    nc.sync.dma_start(wrt[:], wr_d[e])
                    wtiles[e] = (wgt, wut, wdt, wrt)
                wgt, wut, wdt, wrt = wtiles[e]
                xt = xpool.tile([P, HO, w], dt_mm, tag="x")
                nc.sync.dma_start(
                    xt[:], xT_d[e, :, :, off : off + w].rearrange("h p n -> p h n")
                )
                x_live[j] = xt
                at = apool.tile([P, IO, w], dt_mm, tag="act")
                act_tiles[j] = at
                for it in range(IO):
                    g_ps = gps.tile([P, w], mybir.dt.float32, tag="g")
                    u_ps = ups.tile([P, w], mybir.dt.float32, tag="u")
                    for ho in range(HO):
                        nc.tensor.matmul(
                            g_ps[:],
                            wgt[:, ho, it * P : (it + 1) * P],
                            xt[:, ho, :],
                            start=(ho == 0),
                            stop=(ho == HO - 1),
                        )
                    for ho in range(HO):
                        nc.tensor.matmul(
                            u_ps[:],
                            wut[:, ho, it * P : (it + 1) * P],
                            xt[:, ho, :],
                            start=(ho == 0),
                            stop=(ho == HO - 1),
                        )
                    nc.scalar.activation(
                        at[:, it, :], g_ps[:], mybir.ActivationFunctionType.Silu
                    )
                    nc.vector.tensor_mul(at[:, it, :], at[:, it, :], u_ps[:])

            def emit_down(j):
                e, off, w = sched[j]
                _, _, wdt, wrt = wtiles[e]
                at = act_tiles.pop(j)
                for ht in range(HO):
                    d_ps = dps.tile([P, w], mybir.dt.float32, tag="d")
                    for it in range(IO):
                        nc.tensor.matmul(
                            d_ps[:],
                            wdt[:, it, ht * P : (ht + 1) * P],
                            at[:, it, :],
                            start=(it == 0),
                            stop=(it == IO - 1),
                        )
                    ot = opool.tile([P, w], mybir.dt.float32, tag="o")
                    nc.vector.tensor_mul(ot[:], d_ps[:], wrt[:, off : off + w])
                    nc.sync.dma_start(out_d[e, ht, :, off : off + w], ot[:])
                del x_live[j]

            for j in range(len(sched) + 1):
                if j < len(sched):
                    emit_gu(j)
                if j >= 1:
                    emit_down(j - 1)

    nc.compile()
    return nc


def _get_nc(cap: int, repeat: int = 1):
    key = ("nc", cap, repeat)
    if key not in _CACHE:
        _CACHE[key] = _build_nc(cap, repeat)
    return _CACHE[key]


def _get_runner(cap: int, repeat: int = 1):
    """Cached jitted SPMD executor for the kernel (avoids re-tracing per call).

    Mirrors bass2jax.run_bass_via_pjrt's multi-core path, but without output
    donation: this kernel writes every output element, so the result buffers
    don't need to be pre-zeroed, and a non-donating executable can be invoked
    repeatedly on device-resident inputs for timing.
    """
    key = ("runner", cap, repeat)
    if key in _CACHE:
        return _CACHE[key]

    import jax
    from jax.sharding import Mesh, PartitionSpec
    from jax.experimental.shard_map import shard_map
    from concourse import bass2jax, mybir as _mybir

    nc = _get_nc(cap, repeat)
    bass2jax.install_neuronx_cc_hook()

    partition_name = nc.partition_id_tensor.name if nc.partition_id_tensor else None
    in_names, out_names, out_avals, zero_outs = [], [], [], []
    for alloc in nc.m.functions[0].allocations:
        if not isinstance(alloc, _mybir.MemoryLocationSet):
            continue
        name = alloc.memorylocations[0].name
        if alloc.kind == "ExternalInput":
            if name != partition_name:
                in_names.append(name)
        elif alloc.kind == "ExternalOutput":
            out_names.append(name)
            shape = tuple(alloc.tensor_shape)
            dtype = _mybir.dt.np(alloc.dtype)
            out_avals.append(jax.core.ShapedArray(shape, dtype))
            zero_outs.append(np.zeros(shape, dtype))
    n_params = len(in_names)
    all_names = in_names + out_names
    if partition_name is not None:
        all_names = all_names + [partition_name]

    def _body(*args):
        operands = list(args)
        if partition_name is not None:
            operands.append(bass2jax.partition_id_tensor())
        outs = bass2jax._bass_exec_p.bind(
            *operands,
            out_avals=tuple(out_avals),
            in_names=tuple(all_names),
            out_names=tuple(out_names),
            lowering_input_output_aliases=(),
            sim_require_finite=True,
            sim_require_nnan=True,
            nc=nc,
        )
        return tuple(outs)

    devices = jax.devices()[:N_CORES]
    mesh = Mesh(np.asarray(devices), ("core",))
    n_all = n_params + len(out_names)
    sharded = jax.jit(
        shard_map(
            _body,
            mesh=mesh,
            in_specs=(PartitionSpec("core"),) * n_all,
            out_specs=(PartitionSpec("core"),) * len(out_names),
            check_rep=False,
        ),
        keep_unused=True,
    )
    runner = {
        "fn": sharded,
        "in_names": in_names,
        "out_names": out_names,
        "out_avals": out_avals,
        "zero_outs": zero_outs,
    }
    _CACHE[key] = runner
    return runner


def _run_spmd(cap: int, in_maps):
    r = _get_runner(cap)
    concat_in = [
        np.concatenate([np.asarray(m[name]) for m in in_maps], axis=0)
        for name in r["in_names"]
    ]
    concat_zero = [
        np.zeros((N_CORES * z.shape[0], *z.shape[1:]), z.dtype) for z in r["zero_outs"]
    ]
    out_arrs = r["fn"](*concat_in, *concat_zero)
    return [
        {
            name: np.asarray(out_arrs[i]).reshape(N_CORES, *r["out_avals"][i].shape)[c]
            for i, name in enumerate(r["out_names"])
        }
        for c in range(N_CORES)
    ]


def _dispatch(hidden_states, top_k_index, top_k_weights, wg, wu, wd):
    """Host-side routing: sort pairs by expert, pad per-expert groups, build
    per-core input maps. Returns (cap, in_maps, sort_idx, offsets)."""
    hidden_states = np.ascontiguousarray(hidden_states, dtype=np.float32)
    flat_eid = np.asarray(top_k_index, dtype=np.int64).ravel()
    sort_idx = np.argsort(flat_eid, kind="stable")
    tok = sort_idx // K
    counts = np.bincount(flat_eid, minlength=E)
    offsets = np.concatenate(([0], np.cumsum(counts)))

    # smallest 128-multiple capacity covering the largest expert group; the
    # device program is compiled (and cached) per cap value
    cap = max(CHUNK, int(-(-int(counts.max()) // CAP_QUANTUM) * CAP_QUANTUM))

    # sorted, weighted dispatch tensors
    xs_T = np.ascontiguousarray(hidden_states[tok].T)  # [H, T*K] sorted by expert
    w_sorted = np.asarray(top_k_weights, dtype=np.float32).ravel()[sort_idx]

    in_maps = []
    for core in range(N_CORES):
        xT = np.zeros((EL, H, cap), dtype=BF16)
        wr = np.zeros((EL, P, cap), dtype=np.float32)
        for le in range(EL):
            e = core * EL + le
            o0, o1 = offsets[e], offsets[e + 1]
            g = o1 - o0
            xT[le, :, :g] = xs_T[:, o0:o1]
            wr[le, :, :g] = w_sorted[o0:o1][None, :]
        es = slice(core * EL, (core + 1) * EL)
        in_maps.append(
            {
                "xT": xT.reshape(EL, HO, P, cap),
                "wg": _cast_cached(wg, es, (EL, HO, P, I)),
                "wu": _cast_cached(wu, es, (EL, HO, P, I)),
                "wd": _cast_cached(wd, es, (EL, IO, P, H)),
                "wr": wr,
            }
        )
    return cap, in_maps, sort_idx, offsets


def _cast_cached(w, es, shape):
    # bf16 cast of a weight slice, cached on the source array identity (id +
    # cheap fingerprint) so reused weight tensors across calls skip the cast
    w = np.asarray(w)
    fp = (id(w), w.shape, float(w.flat[0]), float(w.flat[w.size // 2 + 1]),
          float(w.flat[w.size - 1]), es.start, es.stop)
    key = ("wcast", fp, shape)
    if key not in _CACHE:
        _CACHE[key] = np.ascontiguousarray(w[es].reshape(shape)).astype(BF16)
    return _CACHE[key]


def kernel(hidden_states, top_k_index, top_k_weights, wg, wu, wd):
    Tn, Hn = hidden_states.shape
    En, _, In = wg.shape
    Kn = top_k_index.shape[1]
    assert (Tn, Hn, En, In, Kn) == (T, H, E, I, K), "kernel hardcoded for spec shapes"

    cap, in_maps, sort_idx, offsets = _dispatch(
        hidden_states, top_k_index, top_k_weights, wg, wu, wd
    )
    results = _run_spmd(cap, in_maps)

    # combine: weighted contributions are already applied on device
    down_sorted = np.empty((T * K, H), dtype=np.float32)
    for core in range(N_CORES):
        o = results[core]["out"].reshape(EL, H, cap)
        for le in range(EL):
            e = core * EL + le
            o0, o1 = offsets[e], offsets[e + 1]
            down_sorted[o0:o1] = o[le, :, : o1 - o0].T

    inv = np.empty(T * K, dtype=np.int64)
    inv[sort_idx] = np.arange(T * K)
    out = down_sorted[inv].reshape(T, K, H).sum(axis=1, dtype=np.float32)
    return out.astype(np.float32)


def measure_hw_ns(inputs, n_rep=5, repeat=5):
    """Amortized per-execution device time (ns): difference between a kernel
    variant that runs the whole schedule `repeat` times in one NEFF and the
    1x kernel, divided by (repeat-1). Launch overhead (~80ms under axon)
    cancels in the difference."""
    import time
    import jax
    from jax.sharding import Mesh, NamedSharding, PartitionSpec

    cap, in_maps, _, _ = _dispatch(**inputs)

    mesh = Mesh(np.asarray(jax.devices()[:N_CORES]), ("core",))
    sh = NamedSharding(mesh, PartitionSpec("core"))

    def timed(rep):
        r = _get_runner(cap, rep)
        concat_in = [
            np.concatenate([np.asarray(m[name]) for m in in_maps], axis=0)
            for name in r["in_names"]
        ]
        concat_zero = [
            np.zeros((N_CORES * z.shape[0], *z.shape[1:]), z.dtype)
            for z in r["zero_outs"]
        ]
        dev_in = [jax.device_put(a, sh) for a in concat_in]
        dev_zero = [jax.device_put(a, sh) for a in concat_zero]
        jax.block_until_ready(r["fn"](*dev_in, *dev_zero))  # warm/compile
        ts = []
        for _ in range(n_rep):
            t0 = time.perf_counter()
            jax.block_until_ready(r["fn"](*dev_in, *dev_zero))
            ts.append(time.perf_counter() - t0)
        return min(ts)

    # interleaved rounds so session drift (thermal/terminal load) cancels
    timed(1)
    timed(repeat)
    slopes = []
    for _ in range(3):
        t1 = timed(1)
        tk = timed(repeat)
        slopes.append((tk - t1) / (repeat - 1) * 1e9)
    slopes.sort()
    return slopes[len(slopes) // 2]



# revision 2
# speedup vs baseline: 1.0151x; 1.0151x over previous
"""Trainium2 Bass kernel for DeepseekV3 naive MoE (expert-parallel over 8 cores).

Contract: kernel(**inputs) takes FULL unsharded numpy inputs
(hidden_states [T,H] f32, top_k_index [T,K] i32, top_k_weights [T,K] f32,
wg [E,H,I] f32, wu [E,H,I] f32, wd [E,I,H] f32) and returns the FULL
[T,H] f32 output, equal to the reference grouped-GEMM MoE.

Strategy (hardcoded for T=8192, H=1024, I=1408, E=32, K=8, 8 cores):
 - Host: replicate tokens K times, stable-sort pairs by expert id. Sort the 32
   expert groups by size; slot j in {0..3} holds size-ranks [8j, 8j+8), one
   expert per core, so every core runs the same schedule with per-slot
   capacity caps[j] = roundup16(max group size in slot j). This cuts padding
   from pad-all-to-global-max (~12.5%) to ~1.5%. Build transposed activations
   xT [H, sum(caps)] per core (contraction dim on partitions), cast matmul
   operands to bf16 (PSUM accumulation stays fp32).
 - Device (SPMD, 4 expert slots per core): per slot, per near-equal column
   chunk (widths kept >=~256 so the per-matmul 128x128 weight reload stays
   hidden behind the matmul stream): gateT = wg.T @ xT, upT = wu.T @ xT
   (PSUM f32 accumulate over H), actT = silu(gateT) * upT (bf16),
   downT = wd.T @ actT, scaled by the per-pair router weight, stored to a
   flat [H, sum(caps)] output.
 - Host: transpose back, unsort, sum the K weighted contributions per token.
"""

import os
import sys

for _p in ("/opt/trn_rl_repo", "/root/.axon_site/_ro/trn_rl_repo"):
    if _p not in sys.path:
        sys.path.insert(0, _p)

# recover wedged NeuronCores on first touch; NEFF cache across rounds
os.environ.setdefault("NEURON_RT_RESET_CORES", "1")
os.environ.setdefault("MYCRO_LOCAL_CACHE", "1")

import numpy as np
import ml_dtypes

import concourse.bass as bass  # noqa: F401  (registers types)
import concourse.tile as tile
from concourse import bacc, mybir

# Problem dims (fixed by the task)
E, H, I, K, T = 32, 1024, 1408, 8, 8192
N_CORES = 8
EL = E // N_CORES  # expert slots per core
P = 128
HO, IO = H // P, I // P  # 8, 11
CHUNK = 512
CAP_QUANTUM = 16


def _chunks_of(cap: int):
    """Split cap columns into near-equal chunks (each <= 512, width a multiple
    of 16, all but possibly the last equal). Equal-ish widths keep every chunk
    wide enough that the per-matmul weight reload (~107ns) stays hidden."""
    n = -(-cap // CHUNK)
    w0 = -(-cap // (n * CAP_QUANTUM)) * CAP_QUANTUM
    out = []
    off = 0
    for _ in range(n):
        w = min(w0, cap - off)
        if w <= 0:
            break
        out.append((off, w))
        off += w
    return out


BF16 = ml_dtypes.bfloat16

_CACHE: dict = {}


def _build_nc(caps: tuple, repeat: int = 1):
    """Build + compile the per-core Bass kernel for per-slot capacities `caps`.

    repeat>1 duplicates the whole schedule in-kernel (same IO); used only to
    amortize launch overhead when measuring device execution time."""
    S = sum(caps)
    bases = [0]
    for c in caps:
        bases.append(bases[-1] + c)
    dt_mm = mybir.dt.bfloat16

    nc = bacc.Bacc("TRN2", target_bir_lowering=False, debug=False)

    xT_d = nc.dram_tensor("xT", [HO, P, S], dt_mm, kind="ExternalInput")
    wg_d = nc.dram_tensor("wg", [EL, HO, P, I], dt_mm, kind="ExternalInput")
    wu_d = nc.dram_tensor("wu", [EL, HO, P, I], dt_mm, kind="ExternalInput")
    wd_d = nc.dram_tensor("wd", [EL, IO, P, H], dt_mm, kind="ExternalInput")
    wr_d = nc.dram_tensor("wr", [P, S], mybir.dt.float32, kind="ExternalInput")
    out_d = nc.dram_tensor("out", [HO, P, S], mybir.dt.float32, kind="ExternalOutput")

    sched = [
        (e, bases[e] + off, w) for e in range(EL) for (off, w) in _chunks_of(caps[e])
    ] * repeat

    with tile.TileContext(nc) as tc:
        with (
            tc.tile_pool(name="wpool", bufs=2) as wpool,
            tc.tile_pool(name="wdpool", bufs=2) as wdpool,
            tc.tile_pool(name="xpool", bufs=3) as xpool,
            tc.tile_pool(name="apool", bufs=2) as apool,
            tc.tile_pool(name="opool", bufs=3) as opool,
            tc.tile_pool(name="rpool", bufs=3) as rpool,
            tc.tile_pool(name="gps", bufs=2, space="PSUM") as gps,
            tc.tile_pool(name="ups", bufs=2, space="PSUM") as ups,
            tc.tile_pool(name="dps", bufs=3, space="PSUM") as dps,
        ):
            wtiles = {}  # live weight tiles for current expert slot
            act_tiles = {}  # chunk index -> act tile
            r_tiles = {}  # chunk index -> router-weight tile
            x_live = {}

            def emit_gu(j):
                e, col, w = sched[j]
                if col == bases[e]:
                    wgt = wpool.tile([P, HO, I], dt_mm, tag="wg")
                    for ho in range(HO):
                        nc.sync.dma_start(wgt[:, ho, :], wg_d[e, ho])
                    wut = wpool.tile([P, HO, I], dt_mm, tag="wu")
                    for ho in range(HO):
                        nc.sync.dma_start(wut[:, ho, :], wu_d[e, ho])
                    wdt = wdpool.tile([P, IO, H], dt_mm, tag="wd")
                    for io in range(IO):
                        nc.sync.dma_start(wdt[:, io, :], wd_d[e, io])
                    wtiles[e] = (wgt, wut, wdt)
                wgt, wut, wdt = wtiles[e]
                xt = xpool.tile([P, HO, w], dt_mm, tag="x")
                nc.sync.dma_start(
                    xt[:], xT_d[:, :, col : col + w].rearrange("h p n -> p h n")
                )
                x_live[j] = xt
                wrt = rpool.tile([P, w], mybir.dt.float32, tag="wr")
                nc.sync.dma_start(wrt[:], wr_d[:, col : col + w])
                r_tiles[j] = wrt
                at = apool.tile([P, IO, w], dt_mm, tag="act")
                act_tiles[j] = at
                for it in range(IO):
                    g_ps = gps.tile([P, w], mybir.dt.float32, tag="g")
                    u_ps = ups.tile([P, w], mybir.dt.float32, tag="u")
                    for ho in range(HO):
                        nc.tensor.matmul(
                            g_ps[:],
                            wgt[:, ho, it * P : (it + 1) * P],
                            xt[:, ho, :],
                            start=(ho == 0),
                            stop=(ho == HO - 1),
                        )
                    for ho in range(HO):
                        nc.tensor.matmul(
                            u_ps[:],
                            wut[:, ho, it * P : (it + 1) * P],
                            xt[:, ho, :],
                            start=(ho == 0),
                            stop=(ho == HO - 1),
                        )
                    nc.scalar.activation(
                        at[:, it, :], g_ps[:], mybir.ActivationFunctionType.Silu
                    )
                    nc.vector.tensor_mul(at[:, it, :], at[:, it, :], u_ps[:])

            def emit_down(j):
                e, col, w = sched[j]
                _, _, wdt = wtiles[e]
                at = act_tiles.pop(j)
                wrt = r_tiles.pop(j)
                for ht in range(HO):
                    d_ps = dps.tile([P, w], mybir.dt.float32, tag="d")
                    for it in range(IO):
                        nc.tensor.matmul(
                            d_ps[:],
                            wdt[:, it, ht * P : (ht + 1) * P],
                            at[:, it, :],
                            start=(it == 0),
                            stop=(it == IO - 1),
                        )
                    ot = opool.tile([P, w], mybir.dt.float32, tag="o")
                    nc.vector.tensor_mul(ot[:], d_ps[:], wrt[:])
                    nc.sync.dma_start(out_d[ht, :, col : col + w], ot[:])
                del x_live[j]

            for j in range(len(sched) + 1):
                if j < len(sched):
                    emit_gu(j)
                if j >= 1:
                    emit_down(j - 1)

    nc.compile()
    return nc


def _get_nc(caps: tuple, repeat: int = 1):
    key = ("nc", caps, repeat)
    if key not in _CACHE:
        _CACHE[key] = _build_nc(caps, repeat)
    return _CACHE[key]


def _get_runner(caps: tuple, repeat: int = 1):
    """Cached jitted SPMD executor for the kernel (avoids re-tracing per call).

    Mirrors bass2jax.run_bass_via_pjrt's multi-core path, but without output
    donation: this kernel writes every output element, so the result buffers
    don't need to be pre-zeroed, and a non-donating executable can be invoked
    repeatedly on device-resident inputs for timing.
    """
    key = ("runner", caps, repeat)
    if key in _CACHE:
        return _CACHE[key]

    import jax
    from jax.sharding import Mesh, PartitionSpec
    from jax.experimental.shard_map import shard_map
    from concourse import bass2jax, mybir as _mybir

    nc = _get_nc(caps, repeat)
    bass2jax.install_neuronx_cc_hook()

    partition_name = nc.partition_id_tensor.name if nc.partition_id_tensor else None
    in_names, out_names, out_avals, zero_outs = [], [], [], []
    for alloc in nc.m.functions[0].allocations:
        if not isinstance(alloc, _mybir.MemoryLocationSet):
            continue
        name = alloc.memorylocations[0].name
        if alloc.kind == "ExternalInput":
            if name != partition_name:
                in_names.append(name)
        elif alloc.kind == "ExternalOutput":
            out_names.append(name)
            shape = tuple(alloc.tensor_shape)
            dtype = _mybir.dt.np(alloc.dtype)
            out_avals.append(jax.core.ShapedArray(shape, dtype))
            zero_outs.append(np.zeros(shape, dtype))
    n_params = len(in_names)
    all_names = in_names + out_names
    if partition_name is not None:
        all_names = all_names + [partition_name]

    def _body(*args):
        operands = list(args)
        if partition_name is not None:
            operands.append(bass2jax.partition_id_tensor())
        outs = bass2jax._bass_exec_p.bind(
            *operands,
            out_avals=tuple(out_avals),
            in_names=tuple(all_names),
            out_names=tuple(out_names),
            lowering_input_output_aliases=(),
            sim_require_finite=True,
            sim_require_nnan=True,
            nc=nc,
        )
        return tuple(outs)

    devices = jax.devices()[:N_CORES]
    mesh = Mesh(np.asarray(devices), ("core",))
    n_all = n_params + len(out_names)
    sharded = jax.jit(
        shard_map(
            _body,
            mesh=mesh,
            in_specs=(PartitionSpec("core"),) * n_all,
            out_specs=(PartitionSpec("core"),) * len(out_names),
            check_rep=False,
        ),
        keep_unused=True,
    )
    runner = {
        "fn": sharded,
        "in_names": in_names,
        "out_names": out_names,
        "out_avals": out_avals,
        "zero_outs": zero_outs,
    }
    _CACHE[key] = runner
    return runner


def _run_spmd(caps: tuple, in_maps):
    r = _get_runner(caps)
    concat_in = [
        np.concatenate([np.asarray(m[name]) for m in in_maps], axis=0)
        for name in r["in_names"]
    ]
    concat_zero = [
        np.zeros((N_CORES * z.shape[0], *z.shape[1:]), z.dtype) for z in r["zero_outs"]
    ]
    out_arrs = r["fn"](*concat_in, *concat_zero)
    return [
        {
            name: np.asarray(out_arrs[i]).reshape(N_CORES, *r["out_avals"][i].shape)[c]
            for i, name in enumerate(r["out_names"])
        }
        for c in range(N_CORES)
    ]


def _dispatch(hidden_states, top_k_index, top_k_weights, wg, wu, wd):
    """Host-side routing: sort pairs by expert, rank experts by group size into
    per-core slots with shared per-slot capacities, build per-core input maps.
    Returns (caps, in_maps, combine_info)."""
    hidden_states = np.ascontiguousarray(hidden_states, dtype=np.float32)
    flat_eid = np.asarray(top_k_index, dtype=np.int64).ravel()
    sort_idx = np.argsort(flat_eid, kind="stable")
    tok = sort_idx // K
    counts = np.bincount(flat_eid, minlength=E)
    offsets = np.concatenate(([0], np.cumsum(counts)))

    # slot j gets the experts ranked [8j, 8j+8) by descending group size (one
    # per core); the shared per-slot capacity is the slot's max size. The
    # device program is compiled (and cached) per caps tuple.
    ranked = np.argsort(-counts, kind="stable")
    assignment = ranked.reshape(EL, N_CORES).T  # [core, slot] -> expert id
    caps = tuple(
        max(
            CAP_QUANTUM,
            int(-(-int(counts[ranked[j * N_CORES]]) // CAP_QUANTUM) * CAP_QUANTUM),
        )
        for j in range(EL)
    )
    S = sum(caps)
    bases = np.concatenate(([0], np.cumsum(caps)))

    # sorted, weighted dispatch tensors
    xs_T = np.ascontiguousarray(hidden_states[tok].T)  # [H, T*K] sorted by expert
    w_sorted = np.asarray(top_k_weights, dtype=np.float32).ravel()[sort_idx]

    in_maps = []
    for core in range(N_CORES):
        xT = np.zeros((H, S), dtype=BF16)
        wr = np.zeros((P, S), dtype=np.float32)
        for slot in range(EL):
            e = int(assignment[core, slot])
            o0, o1 = offsets[e], offsets[e + 1]
            g = o1 - o0
            b = bases[slot]
            xT[:, b : b + g] = xs_T[:, o0:o1]
            wr[:, b : b + g] = w_sorted[o0:o1][None, :]
        es = tuple(int(x) for x in assignment[core])
        in_maps.append(
            {
                "xT": xT.reshape(HO, P, S),
                "wg": _gather_cast(wg, es, (HO, P, I)),
                "wu": _gather_cast(wu, es, (HO, P, I)),
                "wd": _gather_cast(wd, es, (IO, P, H)),
                "wr": wr,
            }
        )
    combine = {
        "sort_idx": sort_idx,
        "offsets": offsets,
        "assignment": assignment,
        "bases": bases,
        "S": S,
    }
    return caps, in_maps, combine


def _gather_cast(w, es, shape):
    # bf16 cast + expert gather of a weight tensor, cached on the source array
    # identity (id + cheap fingerprint) and the expert tuple, so reused weight
    # tensors across calls skip the cast/gather
    w = np.asarray(w)
    fp = (id(w), w.shape, float(w.flat[0]), float(w.flat[w.size // 2 + 1]),
          float(w.flat[w.size - 1]))
    key = ("wgather", fp, es, shape)
    if key not in _CACHE:
        full_key = ("wcast_full", fp, shape)
        if full_key not in _CACHE:
            _CACHE[full_key] = np.ascontiguousarray(w.reshape(E, *shape)).astype(BF16)
        _CACHE[key] = np.ascontiguousarray(_CACHE[full_key][list(es)])
    return _CACHE[key]


def kernel(hidden_states, top_k_index, top_k_weights, wg, wu, wd):
    Tn, Hn = hidden_states.shape
    En, _, In = wg.shape
    Kn = top_k_index.shape[1]
    assert (Tn, Hn, En, In, Kn) == (T, H, E, I, K), "kernel hardcoded for spec shapes"

    caps, in_maps, cb = _dispatch(
        hidden_states, top_k_index, top_k_weights, wg, wu, wd
    )
    results = _run_spmd(caps, in_maps)

    # combine: weighted contributions are already applied on device
    offsets, assignment, bases = cb["offsets"], cb["assignment"], cb["bases"]
    down_sorted = np.empty((T * K, H), dtype=np.float32)
    for core in range(N_CORES):
        o = results[core]["out"].reshape(H, cb["S"])
        for slot in range(EL):
            e = int(assignment[core, slot])
            o0, o1 = offsets[e], offsets[e + 1]
            b = bases[slot]
            down_sorted[o0:o1] = o[:, b : b + (o1 - o0)].T

    inv = np.empty(T * K, dtype=np.int64)
    inv[cb["sort_idx"]] = np.arange(T * K)
    out = down_sorted[inv].reshape(T, K, H).sum(axis=1, dtype=np.float32)
    return out.astype(np.float32)


def measure_hw_ns(inputs, n_rep=5, repeat=5):
    """Amortized per-execution device time (ns): difference between a kernel
    variant that runs the whole schedule `repeat` times in one NEFF and the
    1x kernel, divided by (repeat-1). Launch overhead (~80ms under axon)
    cancels in the difference."""
    import time
    import jax
    from jax.sharding import Mesh, NamedSharding, PartitionSpec

    caps, in_maps, _ = _dispatch(**inputs)

    mesh = Mesh(np.asarray(jax.devices()[:N_CORES]), ("core",))
    sh = NamedSharding(mesh, PartitionSpec("core"))

    def timed(rep):
        r = _get_runner(caps, rep)
        concat_in = [
            np.concatenate([np.asarray(m[name]) for m in in_maps], axis=0)
            for name in r["in_names"]
        ]
        concat_zero = [
            np.zeros((N_CORES * z.shape[0], *z.shape[1:]), z.dtype)
            for z in r["zero_outs"]
        ]
        dev_in = [jax.device_put(a, sh) for a in concat_in]
        dev_zero = [jax.device_put(a, sh) for a in concat_zero]
        jax.block_until_ready(r["fn"](*dev_in, *dev_zero))  # warm/compile
        ts = []
        for _ in range(n_rep):
            t0 = time.perf_counter()
            jax.block_until_ready(r["fn"](*dev_in, *dev_zero))
            ts.append(time.perf_counter() - t0)
        return min(ts)

    # interleaved rounds so session drift (thermal/terminal load) cancels
    timed(1)
    timed(repeat)
    slopes = []
    for _ in range(3):
        t1 = timed(1)
        tk = timed(repeat)
        slopes.append((tk - t1) / (repeat - 1) * 1e9)
    slopes.sort()
    return slopes[len(slopes) // 2]
